# revision 10
# baseline (speedup 1.0000x reference)
"""Trainium2 Bass kernel for nn_CausalPredictor (j-chunk pipeline, bf16).

Reference math (per image y = x[b], all f32):
    zd   = dic @ Wz_w.T + Wz_b                          [K, C]
    att  = softmax((y @ Wy_w.T + Wy_b) @ zd.T * s, k)   [L, K]
    z    = (att * prior) @ dic                          [L, D]
    ly   = y @ cs_w[:, :D].T                            [L, C]
    lz   = z @ cs_w[:, D:].T + cs_b                     [L, C]
    out[i*L+j, c] = ly[i, c] + lz[j, c]                 [L*L, C]

Weight-only algebra is folded on the HOST:
    zdts  = (zd.T + Wz_b) * s                 [C, K]
    M     = Wy_w.T @ zdts                     [D, K]   (bf16 on device)
    ebias = Wy_b @ zdts                       [K]
    gb    = [1 | prior*dic @ csz.T + cs_b]    [K, 1+C] (denom col first,
                                                        cs_b folded into num)

Device graph per core, pipelined at 128-j-chunk granularity, c INNERMOST
everywhere (so every DMA is big contiguous descriptors and the host
assemble is a pure reshape).  y, M, csy are bf16 (verified 3.7e-3 rel err
vs the 2e-2 budget); exp/div run in f32.
    ep    = exp(M.T @ yT_h + ebias)           [K, 512]   8 bf16 mms per half
    per 128-j chunk (j on partitions):
      ndj   = ep_chunk.T @ gb                 [128, 1+C] (den | num), N=22 mm
      lzJ   = ndj[:,1:] * recip(ndj[:,0])     [128, C]   bf16, per-part scalar
      DRAM roundtrip: lzJ -> lzrow[h,jc] -> 0-stride broadcast-load into a
      jc-PAIR rep tile [128, 2, 128, C] (all partitions get the lz image)
    lyI   = (yT_ic.T @ csyT)                  [128, C]   i on partitions, N=21
    lyE   = lyI broadcast along j             [128, 256, C] bf16, once per ic
            (built on GPSIMD; DVE builds the first to cut the ramp)
    ob    = rep + lyE   as FLAT [128, 5376] unit-stride bf16 tensor_add on
            DVE -- rearranged APs satisfy the DVE 2x perf-mode conditions
    DMA ob -> out[ic, h, jp]   (sync queue is output-only; 16 x 1.38 MB,
                                10752 B descriptors)

Perf facts this build is shaped around: PE matmul cost ~ moving columns
(1 cyc/col bf16, 4 f32) + ~280 cyc LoadStationary, so the outer sum is
engine adds, not matmuls; DVE reciprocal is ~9 cyc/elem so the division
runs on [128,1] per-partition scalars; DVE 2x perf mode needs all-2B,
unit-stride, flat APs; HWDGE dma_start semaphore waits execute on the
issuing engine's sequencer, so the lz roundtrip rides the scalar queue
(slack), never the sync queue.

Sharding: 8 cores = 4 images x 2 halves of the i dim, no collectives.
"""

import sys

for _p in ("/opt/trn_rl_repo", "/root/.axon_site/_ro/trn_rl_repo"):
    if _p not in sys.path:
        sys.path.append(_p)

import numpy as np

import concourse.bass as bass
from concourse import bacc
import concourse.mybir as mybir
import concourse.tile as tile
from contextlib import ExitStack

B, L, D, K, C = 4, 1024, 1024, 20, 21
SCALE = 1.0 / float(np.sqrt(np.float32(C)))
F32 = mybir.dt.float32
BF16 = mybir.dt.bfloat16
HALF_L = L // 2          # 512 rows of i per core
N_IC = HALF_L // 128     # 4 i-chunks of 128 per core
N_DC = D // 128          # 8 chunks along the contraction dim
JC = 512                 # j columns per processed half
N_JC = JC // 128         # 4 j-chunks per half
N_JP = 2                 # jc-pairs per half
CW = 128 * C             # 2688 elems per j-chunk per partition
PW = 2 * CW              # 5376 elems per pair unit per partition


def _build_program():
    nc = bacc.Bacc(
        "TRN2",
        target_bir_lowering=False,
        debug=False,
        enable_asserts=False,
        num_devices=8,
    )
    yt_d = nc.dram_tensor("yT", [128, 2, N_DC, N_JC, 128], BF16,
                          kind="ExternalInput").ap()
    m_d = nc.dram_tensor("M", [128, N_DC, K], BF16, kind="ExternalInput").ap()
    csy_d = nc.dram_tensor("csyT", [128, N_DC, C], BF16, kind="ExternalInput").ap()
    gb_d = nc.dram_tensor("gb", [K, C + 1], F32, kind="ExternalInput").ap()
    eb_d = nc.dram_tensor("ebias", [K], F32, kind="ExternalInput").ap()
    lzrow_d = nc.dram_tensor("lzrow", [2, N_JC, CW], BF16, kind="Internal").ap()
    out = nc.dram_tensor("out_loc", [HALF_L, 2, N_JP, PW], BF16,
                         kind="ExternalOutput").ap()

    with tile.TileContext(nc) as tc:
        _emit(tc, out, yt_d, m_d, csy_d, gb_d, eb_d, lzrow_d)
    nc.compile()
    return nc


def _bcast_ap(ap, parts):
    """Partition-broadcast a DRAM AP across `parts` partitions (DMA only)."""
    return bass.AP(tensor=ap.tensor, offset=ap.offset, ap=[[0, parts]] + list(ap.ap))


def _flat(t):
    """[128, a, b(, c)] tile -> flat unit-stride [128, a*b(*c)] AP view."""
    ap = t[:, :, :, :] if len(t.shape) == 4 else t[:, :, :]
    pat = "p a b c -> p (a b c)" if len(t.shape) == 4 else "p a b -> p (a b)"
    return ap.rearrange(pat)


def _emit(tc, out, yt_d, m_d, csy_d, gb_d, eb_d, lzrow_d):
    nc = tc.nc
    ctx = ExitStack()
    with ctx:
        consts = ctx.enter_context(tc.tile_pool(name="consts", bufs=1))
        obpool = ctx.enter_context(tc.tile_pool(name="obpool", bufs=4))
        small = ctx.enter_context(tc.tile_pool(name="small", bufs=3))
        sm_ps = ctx.enter_context(tc.tile_pool(name="sm_ps", bufs=4, space="PSUM"))

        # ---- loads (scalar/ACT HWDGE queue; sync queue is output-only) ----
        yT = [consts.tile([128, N_DC, N_JC, 128], BF16, name=f"yT{h}")
              for h in range(2)]
        nc.scalar.dma_start(out=yT[0], in_=yt_d[:, 0])
        m_sb = consts.tile([128, N_DC, K], BF16, name="m_sb")
        nc.scalar.dma_start(out=m_sb, in_=m_d)
        ebias = consts.tile([K, 1], F32, name="ebias")
        nc.scalar.dma_start(out=ebias, in_=eb_d.unsqueeze(1))
        gb = consts.tile([K, C + 1], F32, name="gb")
        nc.scalar.dma_start(out=gb, in_=gb_d)
        csy_sb = consts.tile([128, N_DC, C], BF16, name="csy_sb")
        nc.scalar.dma_start(out=csy_sb, in_=csy_d)
        nc.scalar.dma_start(out=yT[1], in_=yt_d[:, 1])

        # PE warmup: dependency-free bf16 matmuls so the HAM releases the
        # clock gate before the real (latency-critical) matmuls arrive.
        warm = consts.tile([128, 640], BF16, name="warm")
        nc.vector.memset(warm, 0.0)
        for _ in range(8):
            pw = sm_ps.tile([128, JC], F32, name="pw", tag="sm")
            nc.tensor.matmul(pw, warm[:, 0:128], warm[:, 128:640])

        ep_sb = [consts.tile([K, JC], F32, name=f"ep{h}") for h in range(2)]
        lyI = [consts.tile([128, C], BF16, name=f"lyI{ic}") for ic in range(N_IC)]
        lyE = [consts.tile([128, 2 * 128, C], BF16, name=f"lyE{ic}")
               for ic in range(N_IC)]
        rep = {}

        def ep_half(h):
            """ep[h] = exp(M.T @ yT_half + ebias), bf16 matmuls."""
            ps_u = sm_ps.tile([K, JC], F32, name="sm", tag="sm")
            for dc in range(N_DC):
                nc.tensor.matmul(ps_u, m_sb[:, dc, :], yT[h][:, dc, :, :],
                                 start=(dc == 0), stop=(dc == N_DC - 1))
            nc.scalar.activation(ep_sb[h], ps_u,
                                 mybir.ActivationFunctionType.Exp,
                                 bias=ebias, scale=1.0)

        def lz_chunk(h, jc):
            """lzJ = num/den for 128 j's (j on partitions), DRAM roundtrip
            into slice jc%2 of the (h, jc//2) pair rep tile."""
            jsl = slice(jc * 128, (jc + 1) * 128)
            ps_nd = sm_ps.tile([128, C + 1], F32, name="sm", tag="sm")
            nc.tensor.matmul(ps_nd, ep_sb[h][:, jsl], gb)
            recip = small.tile([128, 1], F32, name="recip", tag="recip")
            nc.vector.reciprocal(recip, ps_nd[:, 0:1])
            lzJ = small.tile([128, C], BF16, name="lzJ", tag="lzJ")
            nc.vector.tensor_scalar_mul(lzJ, ps_nd[:, 1 : C + 1], recip)
            nc.scalar.dma_start(out=lzrow_d[h, jc], in_=lzJ)
            jp, jc2 = jc // 2, jc % 2
            if (h, jp) not in rep:
                rep[(h, jp)] = consts.tile([128, 2, 128, C], BF16,
                                           name=f"rep{h}{jp}")
            nc.scalar.dma_start(
                out=rep[(h, jp)][:, jc2],
                in_=_bcast_ap(lzrow_d[h, jc], 128),
            )

        def chunk_ly(ic, build_eng):
            """lyI[i, c] = (y @ csy)[i, c] (i on partitions, N=21 mms), then
            expand along j -> lyE [128, 256, C]."""
            ps_li = sm_ps.tile([128, C], F32, name="sm", tag="sm")
            for dc in range(N_DC):
                nc.tensor.matmul(ps_li, yT[0][:, dc, ic, :], csy_sb[:, dc, :],
                                 start=(dc == 0), stop=(dc == N_DC - 1))
            nc.scalar.copy(lyI[ic], ps_li)
            build_eng.tensor_copy(
                lyE[ic], lyI[ic].unsqueeze(1).broadcast_to([128, 2 * 128, C]))

        def add_out(h, jp, ic):
            """ob = rep + lyE (flat unit-stride bf16 on DVE), one output DMA."""
            ob = obpool.tile([128, 2, 128, C], BF16, name="ob", tag="ob")
            nc.vector.tensor_add(_flat(ob), _flat(rep[(h, jp)]), _flat(lyE[ic]))
            nc.sync.dma_start(
                out=out[ic * 128 : (ic + 1) * 128, h, jp, :], in_=_flat(ob))

        # ---- schedule: all lz chunks early, adds stream on DVE after ----
        ep_half(0)
        chunk_ly(0, nc.vector)   # DVE builds lyE[0] so adds can start early
        lz_chunk(0, 0)
        lz_chunk(0, 1)
        chunk_ly(1, nc.gpsimd)
        lz_chunk(0, 2)
        lz_chunk(0, 3)
        ep_half(1)
        chunk_ly(2, nc.gpsimd)
        for jc in range(N_JC):
            lz_chunk(1, jc)
        chunk_ly(3, nc.gpsimd)
        for ic in range(N_IC):
            for h in range(2):
                for jp in range(N_JP):
                    add_out(h, jp, ic)


_NC_CACHE = None


def _get_nc():
    global _NC_CACHE
    if _NC_CACHE is None:
        _NC_CACHE = _build_program()
    return _NC_CACHE


def _host_weights(inputs):
    """Fold the weight-only algebra on the host (float64 for headroom)."""
    import ml_dtypes
    dic = np.asarray(inputs["dic"], np.float64)
    prior = np.asarray(inputs["prior"], np.float64)
    wy_w = np.asarray(inputs["Wy_w"], np.float64)
    wy_b = np.asarray(inputs["Wy_b"], np.float64)
    wz_w = np.asarray(inputs["Wz_w"], np.float64)
    wz_b = np.asarray(inputs["Wz_b"], np.float64)
    cs_w = np.asarray(inputs["cs_w"], np.float64)
    cs_b = np.asarray(inputs["cs_b"], np.float64)

    zdts = (wz_w @ dic.T + wz_b[:, None]) * float(SCALE)   # [C, K]
    m = (wy_w.T @ zdts).astype(ml_dtypes.bfloat16)         # [D, K]
    m = np.ascontiguousarray(m.reshape(N_DC, 128, K).transpose(1, 0, 2))
    ebias = (wy_b @ zdts).astype(np.float32)               # [K]
    g = (prior[:, None] * dic) @ cs_w[:, D:].T             # [K, C]
    # col 0 = denominator ones, cols 1..C = numerator weights (cs_b folded)
    gb = np.concatenate([np.ones((K, 1)), g + cs_b[None, :]], axis=1)
    gb = np.ascontiguousarray(gb.astype(np.float32))       # [K, 1+C]
    csyT = cs_w[:, :D].T.astype(ml_dtypes.bfloat16)        # [D, C]
    csyT = np.ascontiguousarray(csyT.reshape(N_DC, 128, C).transpose(1, 0, 2))
    return {"M": m, "csyT": csyT, "gb": gb, "ebias": np.ascontiguousarray(ebias)}


def make_in_maps(inputs):
    import ml_dtypes
    x = np.asarray(inputs["x"], dtype=np.float32)
    w = _host_weights(inputs)
    in_maps = []
    for core in range(8):
        b, ihalf = core % B, core // B
        xT = x[b].T.astype(ml_dtypes.bfloat16)             # [D, L] bf16
        if ihalf:
            xT = np.concatenate([xT[:, HALF_L:], xT[:, :HALF_L]], axis=1)
        yt = np.ascontiguousarray(
            xT.reshape(N_DC, 128, 2, N_JC, 128).transpose(1, 2, 0, 3, 4)
        )                                                  # [128, 2, 8, 4, 128]
        in_maps.append({"yT": yt, **w})
    return in_maps


def assemble(results):
    out = np.empty((B, L, L, C), dtype=np.float32)
    for core in range(8):
        b, ihalf = core % B, core // B
        # device output: [512 i_local, 2 processed-half, 512 j_local, C];
        # processed half 0 covers real j-half `ihalf`, half 1 the other.
        r = results[core]["out_loc"].reshape(HALF_L, 2, JC, C)
        dst = out[b, ihalf * HALF_L : (ihalf + 1) * HALF_L]
        dst[:, ihalf * JC : (ihalf + 1) * JC] = r[:, 0]
        dst[:, (1 - ihalf) * JC : (2 - ihalf) * JC] = r[:, 1]
    return out.reshape(B, L * L, C)


def _install_trace_support():
    """The agent image's antenv lacks axon_hooks, so boot() skipped NTFF hook
    install. Recreate the module and register the ctypes-based hook; also stub
    the S3 artifact upload (no creds in this container)."""
    import types

    if sys.modules.get("antenv.axon_hooks") is None:
        mod = types.ModuleType("antenv.axon_hooks")
        _hook = [None]
        mod.set_axon_ntff_profile_hook = lambda h: _hook.__setitem__(0, h)
        mod.get_axon_ntff_profile_hook = lambda: _hook[0]
        sys.modules["antenv.axon_hooks"] = mod
        import antenv

        antenv.axon_hooks = mod
    import antenv.axon_hooks as ah

    if ah.get_axon_ntff_profile_hook() is None:
        from trn_agent_boot.trn_boot import _ntff_profile_via_ctypes

        ah.set_axon_ntff_profile_hook(
            _ntff_profile_via_ctypes("/opt/axon/libaxon_pjrt.so")
        )
    import concourse.bass_utils as bu

    bu.upload_artifacts = lambda tmpdir: tmpdir


def run(inputs, trace=False, **kw):
    from concourse.bass_utils import run_bass_kernel_spmd

    if trace:
        _install_trace_support()
    nc = _get_nc()
    res = run_bass_kernel_spmd(
        nc, make_in_maps(inputs), core_ids=list(range(8)), trace=trace, **kw
    )
    return assemble(res.results), res


def kernel(**inputs) -> np.ndarray:
    out, _ = run(inputs, trace=False)
    return out
